# revision 11
# baseline (speedup 1.0000x reference)
"""LIF neuron scan kernel for Trainium2 (8 NeuronCores, SPMD).

Reference semantics (per element, scan over T):
    H[t] = V[t-1] - (V[t-1] - 0.5)/2 + x[t]
    S[t] = (H[t] >= 1.0)
    V[t] = S[t] ? 0.5 : H[t]

Kernel formulation (matches the reference, see boundary note):
    g[t] ~= H[t] - 0.5, with
    g[0]   = x[0]
    S[t]   = (g[t] >= 0.5)
    g[t+1] = S[t] ? x[t+1] : 0.5*g[t] + x[t+1]

Engine split per step: the Activation engine computes the spike as
Sign(g - 0.5) -> u8 (f32->u8 conversion saturates -1 to 0, so the result
is {0,1}; verified on HW), which doubles as the DMA'd spike output and
the copy_predicated mask.  The DVE does the two remaining ops of the
serial chain: scalar_tensor_tensor for the decay update and
copy_predicated for the reset.  Spikes stream out as uint8 and are
widened to f32 on the host.  Data-parallel over (B*N) across the 8
cores; no cross-device communication.

Boundary note: Sign gives 0 at g == 0.5 exactly where the reference's
>= gives 1.  P(g == 0.5) ~ 2e-8 per sample; across the 4.3e9 samples a
handful of flips (plus short decay tails) are expected, far inside the
2e-2 relative-error gate.
"""

import sys

import numpy as np

if "/opt/trn_rl_repo" not in sys.path:
    sys.path.insert(0, "/opt/trn_rl_repo")

import bass_rust
import concourse.bass as bass
import concourse.mybir as mybir
import concourse.tile as tile
from concourse.bass_utils import run_bass_kernel_spmd

T, B, N = 64, 32, 32768
NCORES = 8
BN = B * N
PER = BN // NCORES  # 131072 elements per core per timestep
P = 128
F = PER // P  # 1024

_CACHE = {}


def _split_excess_waits(nc: bass.Bass, limit: int = 1) -> None:
    """This walrus codegen rejects any instruction carrying more than one
    sync-wait command.  Move the excess waits onto same-engine NoOps
    inserted immediately before the offending instruction — semantically
    identical, the engine just performs the waits one slot earlier in its
    own stream (one wait per NoOp)."""
    n = 0
    for f in nc.m.functions:
        for blk in f.blocks:
            insts = blk.instructions
            out = []
            for inst in insts:
                si = inst.sync_info
                if si is not None and len(si.on_wait) > limit:
                    waits = list(si.on_wait)
                    excess, keep = waits[:-limit], waits[-limit:]
                    for w in excess:
                        nop = bass_rust.InstNoOp(name=f"I-waitnop-{n}")
                        n += 1
                        nop.engine = inst.engine
                        nop.sync_info = bass_rust.SyncInfo(
                            on_wait=[w], on_update=[]
                        )
                        out.append(nop)
                    si.on_wait = keep
                out.append(inst)
            blk.instructions = out
    return


def _strip_program_order_waits(nc: bass.Bass) -> int:
    """Remove semaphore waits that are implied by same-engine program order.

    Tile conservatively emits @complete waits between same-engine
    instructions (RAW/WAR on recycled tiles).  For full-tile sequential
    sweeps these cannot race: element k of the successor is touched
    ~(F - k) cycles after the predecessor wrote it, far beyond the SBUF
    write-ack latency.  A wait is stripped only when ALL updates to its
    semaphore come from non-DMA instructions of the same engine as the
    waiting instruction (so the hardware's in-order execution already
    guarantees the ordering) and the wait is a monotone >=imm check.
    DMA-queue semaphores are updated at asynchronous transfer completion
    and are never stripped."""
    updaters: dict[int, set] = {}
    dma_sems: set[int] = set()
    for f in nc.m.functions:
        for blk in f.blocks:
            for inst in blk.instructions:
                si = inst.sync_info
                if si is None:
                    continue
                is_dma = "DMA" in type(inst).__name__ or "Dma" in type(inst).__name__
                for u in si.on_update:
                    updaters.setdefault(u.id, set()).add(inst.engine)
                    if is_dma:
                        dma_sems.add(u.id)
    stripped = 0
    for f in nc.m.functions:
        for blk in f.blocks:
            for inst in blk.instructions:
                si = inst.sync_info
                if si is None or not si.on_wait:
                    continue
                keep = []
                for w in si.on_wait:
                    if (
                        w.wait_mode == "sem-ge-imm"
                        and w.wait_reg is None
                        and w.id not in dma_sems
                        and updaters.get(w.id) == {inst.engine}
                    ):
                        stripped += 1
                        continue
                    keep.append(w)
                si.on_wait = keep
    return stripped


def _strip_pred_act_wait(nc: bass.Bass) -> int:
    """Remove copy_predicated's wait on the Activation-engine semaphore.

    Schedule proof: Act's Sign starts right after pred[t-1] completes
    (cross-engine wait kept) and writes mask element k at
    act0 + 185 + 0.833*k ns.  pred[t] follows stt[t] on the DVE, starting
    >= pred[t-1]_end + stt_dur ~ act0 + 1150 ns, and reads mask element k
    at ~1.16*k ns after that.  Slack = 965 + 0.33*k ns >= ~950 ns for
    every element, growing along the sweep, so the mask bytes are always
    long-written before they are read even with no semaphore.  The only
    cross-engine waits that remain order Act after pred[t-1] and the DMAs
    after their producers."""
    act_sems = set()
    other_sems = set()
    for f in nc.m.functions:
        for blk in f.blocks:
            for inst in blk.instructions:
                si = inst.sync_info
                if si is None:
                    continue
                for u in si.on_update:
                    if inst.engine == mybir.EngineType.Activation:
                        act_sems.add(u.id)
                    else:
                        other_sems.add(u.id)
    act_only = act_sems - other_sems
    stripped = 0
    for f in nc.m.functions:
        for blk in f.blocks:
            for inst in blk.instructions:
                if type(inst).__name__ != "InstCopyPredicated":
                    continue
                si = inst.sync_info
                if si is None or not si.on_wait:
                    continue
                keep = [w for w in si.on_wait if w.id not in act_only]
                stripped += len(si.on_wait) - len(keep)
                si.on_wait = keep
    return stripped


def build_nc() -> bass.Bass:
    nc = bass.Bass(enable_partition_id=False)
    f32 = mybir.dt.float32
    u8 = mybir.dt.uint8
    Alu = mybir.AluOpType
    Act = mybir.ActivationFunctionType
    x = nc.dram_tensor("x", [T, P, F], f32, kind="ExternalInput")
    s = nc.dram_tensor("s", [T, P, F], u8, kind="ExternalOutput")

    with tile.TileContext(nc) as tc:
        with (
            tc.tile_pool(name="xin", bufs=3) as xpool,
            tc.tile_pool(name="g", bufs=4) as gpool,
            tc.tile_pool(name="sout", bufs=20) as spool,
        ):
            bneg = gpool.tile([P, 1], f32, tag="bneg")
            nc.vector.memset(bneg[:], -0.5)
            xn = xpool.tile([P, F], f32)
            nc.sync.dma_start(xn[:], x[0])
            # g[0] = x[0]: alias the freshly-DMA'd tile, no copy needed.
            g = xn
            for t in range(T):
                st = spool.tile([P, F], u8)
                nc.scalar.activation(st[:], g[:], Act.Sign, bias=bneg[:], scale=1.0)
                nc.sync.dma_start(s[t], st[:])
                if t + 1 < T:
                    xn = xpool.tile([P, F], f32)
                    nc.sync.dma_start(xn[:], x[t + 1])
                    a = gpool.tile([P, F], f32, tag="a")
                    nc.vector.scalar_tensor_tensor(
                        a[:], g[:], 0.5, xn[:], Alu.mult, Alu.add
                    )
                    nc.vector.copy_predicated(a[:], st[:], xn[:])
                    g = a
    _strip_program_order_waits(nc)
    _strip_pred_act_wait(nc)
    _split_excess_waits(nc)
    return nc


def _get_nc() -> bass.Bass:
    if "nc" not in _CACHE:
        _CACHE["nc"] = build_nc()
    return _CACHE["nc"]


def kernel(x: np.ndarray, **run_kwargs):
    x = np.asarray(x)
    assert x.shape == (T, B, N), x.shape
    assert x.dtype == np.float32, x.dtype
    xf = x.reshape(T, BN)
    in_maps = [
        {"x": np.ascontiguousarray(xf[:, k * PER : (k + 1) * PER]).reshape(T, P, F)}
        for k in range(NCORES)
    ]
    res = run_bass_kernel_spmd(_get_nc(), in_maps, list(range(NCORES)), **run_kwargs)
    out = np.empty((T, BN), dtype=np.float32)
    for k in range(NCORES):
        out[:, k * PER : (k + 1) * PER] = (
            np.asarray(res.results[k]["s"]).reshape(T, PER).astype(np.float32)
        )
    out = out.reshape(T, B, N)
    if run_kwargs:
        return out, res
    return out


# revision 12
# speedup vs baseline: 1.1923x; 1.1923x over previous
"""LIF neuron scan kernel for Trainium2 (8 NeuronCores, SPMD).

Reference semantics (per element, scan over T):
    H[t] = V[t-1] - (V[t-1] - 0.5)/2 + x[t]
    S[t] = (H[t] >= 1.0)
    V[t] = S[t] ? 0.5 : H[t]

Kernel formulation (matches the reference, see boundary note):
    g[t] ~= H[t] - 0.5, with
    g[0]   = x[0]
    S[t]   = (g[t] >= 0.5)
    g[t+1] = S[t] ? x[t+1] : 0.5*g[t] + x[t+1]

Engine split per step: the Activation engine computes the spike as
Sign(g - 0.5) -> u8 (f32->u8 conversion saturates -1 to 0, so the result
is {0,1}; verified on HW), which doubles as the DMA'd spike output and
the copy_predicated mask.  The DVE does the two remaining ops of the
serial chain: scalar_tensor_tensor for the decay update and
copy_predicated for the reset.  Spikes stream out as uint8 and are
widened to f32 on the host.  Data-parallel over (B*N) across the 8
cores; no cross-device communication.

Boundary note: Sign gives 0 at g == 0.5 exactly where the reference's
>= gives 1.  P(g == 0.5) ~ 2e-8 per sample; across the 4.3e9 samples a
handful of flips (plus short decay tails) are expected, far inside the
2e-2 relative-error gate.
"""

import sys

import numpy as np

if "/opt/trn_rl_repo" not in sys.path:
    sys.path.insert(0, "/opt/trn_rl_repo")

import bass_rust
import concourse.bass as bass
import concourse.mybir as mybir
import concourse.tile as tile
from concourse.bass_utils import run_bass_kernel_spmd

T, B, N = 64, 32, 32768
NCORES = 8
BN = B * N
PER = BN // NCORES  # 131072 elements per core per timestep
P = 128
F = PER // P  # 1024

_CACHE = {}


def _split_excess_waits(nc: bass.Bass, limit: int = 1) -> None:
    """This walrus codegen rejects any instruction carrying more than one
    sync-wait command.  Move the excess waits onto same-engine NoOps
    inserted immediately before the offending instruction — semantically
    identical, the engine just performs the waits one slot earlier in its
    own stream (one wait per NoOp)."""
    n = 0
    for f in nc.m.functions:
        for blk in f.blocks:
            insts = blk.instructions
            out = []
            for inst in insts:
                si = inst.sync_info
                if si is not None and len(si.on_wait) > limit:
                    waits = list(si.on_wait)
                    excess, keep = waits[:-limit], waits[-limit:]
                    for w in excess:
                        nop = bass_rust.InstNoOp(name=f"I-waitnop-{n}")
                        n += 1
                        nop.engine = inst.engine
                        nop.sync_info = bass_rust.SyncInfo(
                            on_wait=[w], on_update=[]
                        )
                        out.append(nop)
                    si.on_wait = keep
                out.append(inst)
            blk.instructions = out
    return


def _strip_program_order_waits(nc: bass.Bass) -> int:
    """Remove semaphore waits that are implied by same-engine program order.

    Tile conservatively emits @complete waits between same-engine
    instructions (RAW/WAR on recycled tiles).  For full-tile sequential
    sweeps these cannot race: element k of the successor is touched
    ~(F - k) cycles after the predecessor wrote it, far beyond the SBUF
    write-ack latency.  A wait is stripped only when ALL updates to its
    semaphore come from non-DMA instructions of the same engine as the
    waiting instruction (so the hardware's in-order execution already
    guarantees the ordering) and the wait is a monotone >=imm check.
    DMA-queue semaphores are updated at asynchronous transfer completion
    and are never stripped."""
    updaters: dict[int, set] = {}
    dma_sems: set[int] = set()
    for f in nc.m.functions:
        for blk in f.blocks:
            for inst in blk.instructions:
                si = inst.sync_info
                if si is None:
                    continue
                is_dma = "DMA" in type(inst).__name__ or "Dma" in type(inst).__name__
                for u in si.on_update:
                    updaters.setdefault(u.id, set()).add(inst.engine)
                    if is_dma:
                        dma_sems.add(u.id)
    stripped = 0
    for f in nc.m.functions:
        for blk in f.blocks:
            for inst in blk.instructions:
                si = inst.sync_info
                if si is None or not si.on_wait:
                    continue
                keep = []
                for w in si.on_wait:
                    if (
                        w.wait_mode == "sem-ge-imm"
                        and w.wait_reg is None
                        and w.id not in dma_sems
                        and updaters.get(w.id) == {inst.engine}
                    ):
                        stripped += 1
                        continue
                    keep.append(w)
                si.on_wait = keep
    return stripped


def _strip_pred_act_wait(nc: bass.Bass) -> int:
    """Remove copy_predicated's wait on the Activation-engine semaphore.

    Schedule proof: Act's Sign starts right after pred[t-1] completes
    (cross-engine wait kept) and writes mask element k at
    act0 + 185 + 0.833*k ns.  pred[t] follows stt[t] on the DVE, starting
    >= pred[t-1]_end + stt_dur ~ act0 + 1150 ns, and reads mask element k
    at ~1.16*k ns after that.  Slack = 965 + 0.33*k ns >= ~950 ns for
    every element, growing along the sweep, so the mask bytes are always
    long-written before they are read even with no semaphore.  The only
    cross-engine waits that remain order Act after pred[t-1] and the DMAs
    after their producers."""
    act_sems = set()
    other_sems = set()
    for f in nc.m.functions:
        for blk in f.blocks:
            for inst in blk.instructions:
                si = inst.sync_info
                if si is None:
                    continue
                for u in si.on_update:
                    if inst.engine == mybir.EngineType.Activation:
                        act_sems.add(u.id)
                    else:
                        other_sems.add(u.id)
    act_only = act_sems - other_sems
    stripped = 0
    for f in nc.m.functions:
        for blk in f.blocks:
            for inst in blk.instructions:
                if type(inst).__name__ != "InstCopyPredicated":
                    continue
                si = inst.sync_info
                if si is None or not si.on_wait:
                    continue
                keep = [w for w in si.on_wait if w.id not in act_only]
                stripped += len(si.on_wait) - len(keep)
                si.on_wait = keep
    return stripped


def build_nc() -> bass.Bass:
    nc = bass.Bass(enable_partition_id=False)
    f32 = mybir.dt.float32
    u8 = mybir.dt.uint8
    Alu = mybir.AluOpType
    Act = mybir.ActivationFunctionType
    x = nc.dram_tensor("x", [T, P, F], f32, kind="ExternalInput")
    s = nc.dram_tensor("s", [T, P, F], u8, kind="ExternalOutput")

    with tile.TileContext(nc) as tc:
        with (
            tc.tile_pool(name="xin", bufs=10) as xpool,
            tc.tile_pool(name="g", bufs=4) as gpool,
            tc.tile_pool(name="sout", bufs=20) as spool,
        ):
            bneg = gpool.tile([P, 1], f32, tag="bneg")
            nc.vector.memset(bneg[:], -0.5)
            xn = xpool.tile([P, F], f32)
            nc.sync.dma_start(xn[:], x[0])
            # g[0] = x[0]: alias the freshly-DMA'd tile, no copy needed.
            g = xn
            for t in range(T):
                st = spool.tile([P, F], u8)
                nc.scalar.activation(st[:], g[:], Act.Sign, bias=bneg[:], scale=1.0)
                nc.sync.dma_start(s[t], st[:])
                if t + 1 < T:
                    xn = xpool.tile([P, F], f32)
                    nc.sync.dma_start(xn[:], x[t + 1])
                    a = gpool.tile([P, F], f32, tag="a")
                    nc.vector.scalar_tensor_tensor(
                        a[:], g[:], 0.5, xn[:], Alu.mult, Alu.add
                    )
                    nc.vector.copy_predicated(a[:], st[:], xn[:])
                    g = a
    _strip_program_order_waits(nc)
    _strip_pred_act_wait(nc)
    _split_excess_waits(nc)
    return nc


def _get_nc() -> bass.Bass:
    if "nc" not in _CACHE:
        _CACHE["nc"] = build_nc()
    return _CACHE["nc"]


def kernel(x: np.ndarray, **run_kwargs):
    x = np.asarray(x)
    assert x.shape == (T, B, N), x.shape
    assert x.dtype == np.float32, x.dtype
    xf = x.reshape(T, BN)
    in_maps = [
        {"x": np.ascontiguousarray(xf[:, k * PER : (k + 1) * PER]).reshape(T, P, F)}
        for k in range(NCORES)
    ]
    res = run_bass_kernel_spmd(_get_nc(), in_maps, list(range(NCORES)), **run_kwargs)
    out = np.empty((T, BN), dtype=np.float32)
    for k in range(NCORES):
        out[:, k * PER : (k + 1) * PER] = (
            np.asarray(res.results[k]["s"]).reshape(T, PER).astype(np.float32)
        )
    out = out.reshape(T, B, N)
    if run_kwargs:
        return out, res
    return out
